# revision 1
# baseline (speedup 1.0000x reference)
import numpy as np

# ---- static problem config (hardcoded per spec: scores (1,128,128,18)) ----
STRIDE = 8
H = W = 128
NUM_ANCHORS = 9
N = H * W * NUM_ANCHORS          # 147456 anchors
N_CORES = 8
NS = N // N_CORES                # 18432 anchors per core
NEG_OVERLAP = 0.3
POS_OVERLAP = 0.7


def _generate_anchors(base_size=16, ratios=(0.5, 1.0, 2.0), scales=(8.0, 16.0, 32.0)):
    ratios = np.asarray(ratios, np.float64)
    scales = np.asarray(scales, np.float64)
    w = h = float(base_size)
    x_ctr = 0.5 * (w - 1.0)
    y_ctr = 0.5 * (h - 1.0)
    size = w * h
    ws_r = np.round(np.sqrt(size / ratios))
    hs_r = np.round(ws_r * ratios)
    ws = (ws_r[:, None] * scales[None, :]).reshape(-1)
    hs = (hs_r[:, None] * scales[None, :]).reshape(-1)
    anchors = np.stack([x_ctr - 0.5 * (ws - 1.0),
                        y_ctr - 0.5 * (hs - 1.0),
                        x_ctr + 0.5 * (ws - 1.0),
                        y_ctr + 0.5 * (hs - 1.0)], axis=1)
    return anchors.astype(np.float32)


def _shifted_anchors(height, width, stride):
    base = _generate_anchors()
    sx = np.arange(width, dtype=np.float32) * stride
    sy = np.arange(height, dtype=np.float32) * stride
    sx, sy = np.meshgrid(sx, sy)
    shifts = np.stack([sx.ravel(), sy.ravel(), sx.ravel(), sy.ravel()], axis=1)
    all_anchors = shifts[:, None, :] + base[None, :, :]
    return all_anchors.reshape(-1, 4)


_ANCHORS = _shifted_anchors(H, W, STRIDE)  # (N,4) float32, static

_PMAP_FN = None


def _get_pmap_fn():
    """Device function: each of the 8 cores gets an anchor slab (NS,4) and the
    replicated gt boxes; computes its IoU slab, per-anchor max/argmax, bbox
    regression targets, and the local per-gt max/argmax (combined on host)."""
    global _PMAP_FN
    if _PMAP_FN is not None:
        return _PMAP_FN
    import jax
    import jax.numpy as jnp
    from functools import partial

    @partial(jax.pmap, axis_name="d", devices=jax.devices()[:N_CORES])
    def fn(anchors, gt):
        # anchors (NS,4), gt (K,4)
        area_a = (anchors[:, 2] - anchors[:, 0] + 1.0) * (anchors[:, 3] - anchors[:, 1] + 1.0)
        area_g = (gt[:, 2] - gt[:, 0] + 1.0) * (gt[:, 3] - gt[:, 1] + 1.0)
        iw = jnp.maximum(jnp.minimum(anchors[:, None, 2], gt[None, :, 2])
                         - jnp.maximum(anchors[:, None, 0], gt[None, :, 0]) + 1.0, 0.0)
        ih = jnp.maximum(jnp.minimum(anchors[:, None, 3], gt[None, :, 3])
                         - jnp.maximum(anchors[:, None, 1], gt[None, :, 1]) + 1.0, 0.0)
        inter = iw * ih
        overlaps = inter / (area_a[:, None] + area_g[None, :] - inter)  # (NS,K)

        argmax_inds = jnp.argmax(overlaps, axis=1)       # (NS,)
        max_overlaps = jnp.max(overlaps, axis=1)         # (NS,)
        gt_max = jnp.max(overlaps, axis=0)               # (K,) local
        gt_arg = jnp.argmax(overlaps, axis=0)            # (K,) local (first-win)

        matched = gt[argmax_inds]                        # (NS,4)
        ew = anchors[:, 2] - anchors[:, 0] + 1.0
        eh = anchors[:, 3] - anchors[:, 1] + 1.0
        ecx = anchors[:, 0] + 0.5 * ew
        ecy = anchors[:, 1] + 0.5 * eh
        gw = matched[:, 2] - matched[:, 0] + 1.0
        gh = matched[:, 3] - matched[:, 1] + 1.0
        gcx = matched[:, 0] + 0.5 * gw
        gcy = matched[:, 1] + 0.5 * gh
        targets = jnp.stack([(gcx - ecx) / ew,
                             (gcy - ecy) / eh,
                             jnp.log(gw / ew),
                             jnp.log(gh / eh)], axis=1)  # (NS,4)
        return max_overlaps, gt_max, gt_arg.astype(jnp.int32), targets

    _PMAP_FN = fn
    return fn


def _host_reference(scores, im_info, gt_boxes):
    """Pure-numpy fallback (exact same math)."""
    a = _ANCHORS
    gt = np.asarray(gt_boxes[0], np.float32)
    area_a = (a[:, 2] - a[:, 0] + 1.0) * (a[:, 3] - a[:, 1] + 1.0)
    area_g = (gt[:, 2] - gt[:, 0] + 1.0) * (gt[:, 3] - gt[:, 1] + 1.0)
    iw = np.maximum(np.minimum(a[:, None, 2], gt[None, :, 2])
                    - np.maximum(a[:, None, 0], gt[None, :, 0]) + 1.0, 0.0)
    ih = np.maximum(np.minimum(a[:, None, 3], gt[None, :, 3])
                    - np.maximum(a[:, None, 1], gt[None, :, 1]) + 1.0, 0.0)
    inter = (iw * ih).astype(np.float32)
    overlaps = inter / (area_a[:, None] + area_g[None, :] - inter)
    argmax_inds = np.argmax(overlaps, axis=1)
    max_overlaps = np.max(overlaps, axis=1)
    gt_argmax_inds = np.argmax(overlaps, axis=0)
    matched = gt[argmax_inds]
    ew = a[:, 2] - a[:, 0] + 1.0
    eh = a[:, 3] - a[:, 1] + 1.0
    ecx = a[:, 0] + 0.5 * ew
    ecy = a[:, 1] + 0.5 * eh
    gw = matched[:, 2] - matched[:, 0] + 1.0
    gh = matched[:, 3] - matched[:, 1] + 1.0
    gcx = matched[:, 0] + 0.5 * gw
    gcy = matched[:, 1] + 0.5 * gh
    targets = np.stack([(gcx - ecx) / ew, (gcy - ecy) / eh,
                        np.log(gw / ew), np.log(gh / eh)], axis=1).astype(np.float32)
    return max_overlaps, gt_argmax_inds, targets


def _assemble_labels(max_overlaps, gt_argmax_inds, im_info):
    labels = np.full((N,), -1.0, np.float32)
    labels[max_overlaps < NEG_OVERLAP] = 0.0
    um = np.zeros((N,), np.int64)
    np.add.at(um, gt_argmax_inds, 1)
    labels[um == 1] = 1.0
    labels[max_overlaps >= POS_OVERLAP] = 1.0
    img_h = float(np.asarray(im_info)[0, 0])
    img_w = float(np.asarray(im_info)[0, 1])
    a = _ANCHORS
    inside = ((a[:, 0] >= 0.0) & (a[:, 1] >= 0.0)
              & (a[:, 2] < img_w) & (a[:, 3] < img_h))
    labels[~inside] = -1.0
    return labels


def kernel(scores, im_info, gt_boxes):
    gt = np.asarray(gt_boxes, np.float32)[0]             # (K,4)
    try:
        fn = _get_pmap_fn()
        anchors_sh = _ANCHORS.reshape(N_CORES, NS, 4)
        gt_rep = np.broadcast_to(gt, (N_CORES,) + gt.shape)
        max_ov, gt_max, gt_arg, targets = fn(anchors_sh, gt_rep)
        max_overlaps = np.asarray(max_ov).reshape(N)
        targets = np.asarray(targets).reshape(N, 4)
        gt_max = np.asarray(gt_max)                      # (8,K)
        gt_arg = np.asarray(gt_arg).astype(np.int64)     # (8,K)
        # cross-core argmax combine: smallest global index achieving global max
        gmax = gt_max.max(axis=0)                        # (K,)
        offs = (np.arange(N_CORES, dtype=np.int64) * NS)[:, None]
        cand = np.where(gt_max == gmax[None, :], gt_arg + offs, N)
        gt_argmax_inds = cand.min(axis=0)                # (K,)
    except Exception:
        max_overlaps, gt_argmax_inds, targets = _host_reference(scores, im_info, gt_boxes)

    labels = _assemble_labels(max_overlaps, gt_argmax_inds, im_info)
    return labels.astype(np.float32), targets.astype(np.float32)
